# revision 1
# baseline (speedup 1.0000x reference)
"""3-layer GCN (CrystalGCN) on 8 TRN2 NeuronCores.

Strategy (graph/data parallel, nodes sharded by range):
  - 50000 nodes -> 6250/core (padded 6272 = 49 tiles of 128).
  - Edges (incl. self-loops) assigned to the core owning their dst.
  - Per layer l: z = (A_hat @ h) @ W + b  (associativity: aggregate first).
    * gather h[src] rows via gpsimd.dma_gather (bf16, sorted by src,
      lo/hi table split to fit int16 indices),
    * scatter-add via PE matmul: aggT[f, dst] += msgs[e, f].T @ S[e, dst],
      S one-hot * dinv[dst] built on DVE (iota is_equal + mult),
    * dense matmul aggT.T @ W accumulating in PSUM,
    * epilogue relu(z+b) * dinv (pre-scales next layer's gather source).
  - AllGather replicates h between layers; final log_softmax on-chip.

Host preprocessing (numpy) builds index/metadata tensors; the device
kernel is static given the (fixed) edge distribution statistics.
"""
import numpy as np
import ml_dtypes

N = 50000
E = 800000
F_IN, F_HID, F_OUT = 128, 256, 10
F_OUT_P = 16
NCORES = 8
NSH = N // NCORES            # 6250
P = 128
NT = (NSH + P - 1) // P      # 49 node tiles per core
NSHP = NT * P                # 6272 padded shard rows
NROWS = NSHP * NCORES        # 50176 padded global rows
HI_OFF = 17408               # hi table = rows [17408, 50176), 32768 rows
LO_LIM = 32768
BATCH = 4                    # node tiles per gather pair

BF16 = ml_dtypes.bfloat16


def _wrap_idx16(vals):
    """dma_gather index layout: edge i -> [i%16, i//16], replicated to 8
    groups of 16 partitions (one copy per Q7 core)."""
    n = len(vals)
    assert n % 16 == 0
    blk = np.asarray(vals, dtype=np.int16).reshape(n // 16, 16).T
    return np.tile(blk, (8, 1))


def _preprocess(x, edge_index):
    """Build per-core gather/scatter metadata. Returns dict of host arrays."""
    x = np.asarray(x, dtype=np.float32)
    ei = np.asarray(edge_index, dtype=np.int64)
    src_all = np.concatenate([ei[0], np.arange(N, dtype=np.int64)])
    dst_all = np.concatenate([ei[1], np.arange(N, dtype=np.int64)])
    deg = np.bincount(dst_all, minlength=N).astype(np.float32)
    dinv = np.where(deg > 0, 1.0 / np.sqrt(deg), 0.0).astype(np.float32)

    # gather row numbering uses padded global rows
    gidx_all = (src_all // NSH) * NSHP + (src_all % NSH)

    # x gather source: dinv-prescaled, padded layout, bf16
    x_pad = np.zeros((NROWS, F_IN), dtype=BF16)
    xs = (x * dinv[:, None]).astype(BF16)
    for c in range(NCORES):
        x_pad[c * NSHP:c * NSHP + NSH] = xs[c * NSH:(c + 1) * NSH]

    core_of = dst_all // NSH
    tile_of = (dst_all % NSH) // P
    slot_of = (dst_all % NSH) % P

    # per (core, tile) edge lists sorted by gidx
    per = {}
    cnts = np.zeros((NCORES, NT), dtype=np.int64)
    lo_cap = np.zeros((NCORES, NT), dtype=np.int64)
    hi_cap = np.zeros((NCORES, NT), dtype=np.int64)
    order = np.lexsort((gidx_all, tile_of, core_of))
    g_sorted = gidx_all[order]
    slot_sorted = slot_of[order]
    dst_sorted = dst_all[order]
    key = core_of[order] * NT + tile_of[order]
    starts = np.searchsorted(key, np.arange(NCORES * NT))
    ends = np.searchsorted(key, np.arange(NCORES * NT), side="right")
    for c in range(NCORES):
        for t in range(NT):
            k = c * NT + t
            g = g_sorted[starts[k]:ends[k]]
            per[(c, t)] = (g, slot_sorted[starts[k]:ends[k]],
                           dst_sorted[starts[k]:ends[k]])
            cnts[c, t] = len(g)
            lo_cap[c, t] = np.searchsorted(g, LO_LIM)
            hi_cap[c, t] = len(g) - np.searchsorted(g, HI_OFF)

    tl_min = int(np.ceil((cnts - hi_cap).max() / P))
    tl_max = int(lo_cap.min() // P)
    assert tl_min <= tl_max, (tl_min, tl_max)
    # TL=8 makes each lo stream an exact multiple of the 1024-idx
    # dma_gather limit (8 tiles * 128)
    TL = int(np.clip(8, tl_min, tl_max))
    TH = int(np.ceil((cnts.max() - TL * P) / P))
    TNT = TL + TH

    batches = [list(range(i, min(i + BATCH, NT))) for i in range(0, NT, BATCH)]

    cores = []
    for c in range(NCORES):
        dslot = np.zeros((P, NT * TNT), dtype=np.float32)
        dinvd = np.zeros((P, NT * TNT), dtype=np.float32)
        lo_idx_parts = []
        hi_idx_parts = []
        for batch in batches:
            lo_stream = np.zeros(len(batch) * TL * P, dtype=np.int64)
            hi_stream = np.zeros(len(batch) * TH * P, dtype=np.int64)
            for bi, t in enumerate(batch):
                g, sl, dd = per[(c, t)]
                nlo = TL * P
                glo, ghi = g[:nlo], g[nlo:]
                sllo, slhi = sl[:nlo], sl[nlo:]
                ddlo, ddhi = dd[:nlo], dd[nlo:]
                assert glo.max() < LO_LIM and (len(ghi) == 0 or ghi.min() >= HI_OFF)
                lo_stream[bi * nlo:(bi + 1) * nlo] = glo
                nhi = len(ghi)
                hi_stream[bi * TH * P:bi * TH * P + nhi] = ghi - HI_OFF
                # hi dummies stay idx 0 (valid row, dinv 0)
                # metadata: tile j of node-tile t is meta col t*TNT+j
                for m in range(nlo):
                    pass  # vectorized below
                mlo = np.arange(nlo)
                dslot[mlo % P, t * TNT + mlo // P] = sllo
                dinvd[mlo % P, t * TNT + mlo // P] = dinv[ddlo]
                mhi = np.arange(nhi)
                dslot[mhi % P, t * TNT + TL + mhi // P] = slhi
                dinvd[mhi % P, t * TNT + TL + mhi // P] = dinv[ddhi]
            lo_idx_parts.append(_wrap_idx16(lo_stream))
            hi_idx_parts.append(_wrap_idx16(hi_stream - 0))
        idx_lo = np.concatenate(lo_idx_parts, axis=1)
        idx_hi = np.concatenate(hi_idx_parts, axis=1)

        dinv_node = np.zeros((P, NT), dtype=np.float32)
        loc = np.arange(NSH)
        dinv_node[loc % P, loc // P] = dinv[c * NSH:(c + 1) * NSH]

        iota = np.broadcast_to(np.arange(P, dtype=np.float32), (P, P))
        meta = np.concatenate([iota, dslot, dinvd, dinv_node], axis=1).astype(np.float32)
        cores.append({"idx_lo": idx_lo, "idx_hi": idx_hi, "meta": meta})

    return {
        "x_pad": x_pad, "cores": cores, "TL": TL, "TH": TH, "TNT": TNT,
        "batches": batches,
    }


def _build_program(TL, TH, TNT, batches):
    import concourse.bass as bass
    from concourse import bacc
    import concourse.mybir as mybir
    from concourse.tile import TileContext

    dt = mybir.dt
    Alu = mybir.AluOpType
    Act = mybir.ActivationFunctionType
    TE = NT * TNT

    nc = bacc.Bacc(num_devices=NCORES)
    x_pad = nc.dram_tensor("x_pad", [NROWS, F_IN], dt.bfloat16, kind="ExternalInput")
    idx_lo = nc.dram_tensor("idx_lo", [P, NT * TL * 8], dt.int16, kind="ExternalInput")
    idx_hi = nc.dram_tensor("idx_hi", [P, NT * TH * 8], dt.int16, kind="ExternalInput")
    meta = nc.dram_tensor("meta", [P, P + 2 * TE + NT], dt.float32, kind="ExternalInput")
    wts = nc.dram_tensor("wts", [P, 256 + 512 + 32], dt.bfloat16, kind="ExternalInput")
    bias = nc.dram_tensor("bias", [P, 256 + 256 + 16], dt.float32, kind="ExternalInput")
    out_d = nc.dram_tensor("out", [NSHP, F_OUT_P], dt.float32, kind="ExternalOutput")

    with TileContext(nc) as tc:
        with tc.tile_pool(name="const", bufs=1) as cpool, \
             tc.tile_pool(name="msgs", bufs=2) as mpool, \
             tc.tile_pool(name="work", bufs=3) as wpool, \
             tc.tile_pool(name="big", bufs=1) as bigpool, \
             tc.tile_pool(name="ps", bufs=2, space="PSUM") as pspool, \
             tc.tile_pool(name="dram", bufs=1, space="DRAM") as dpool:

            idxlo_sb = cpool.tile([P, NT * TL * 8], dt.int16)
            nc.sync.dma_start(out=idxlo_sb[:], in_=idx_lo[:])
            idxhi_sb = cpool.tile([P, NT * TH * 8], dt.int16)
            nc.sync.dma_start(out=idxhi_sb[:], in_=idx_hi[:])
            meta_sb = cpool.tile([P, P + 2 * TE + NT], dt.float32)
            nc.sync.dma_start(out=meta_sb[:], in_=meta[:])
            wts_sb = cpool.tile([P, 256 + 512 + 32], dt.bfloat16)
            nc.sync.dma_start(out=wts_sb[:], in_=wts[:])
            bias_sb = cpool.tile([P, 256 + 256 + 16], dt.float32)
            nc.sync.dma_start(out=bias_sb[:], in_=bias[:])

            iota_ap = meta_sb[:, 0:P]
            dslot0 = P
            dinvd0 = P + TE
            dinvn0 = P + 2 * TE

            h1_shard = dpool.tile([NSHP, F_HID], dt.bfloat16)
            h2_shard = dpool.tile([NSHP, F_HID], dt.bfloat16)
            h1_full = dpool.tile([NROWS, F_HID], dt.bfloat16, addr_space="Shared")
            h2_full = dpool.tile([NROWS, F_HID], dt.bfloat16, addr_space="Shared")

            w_chunks = {
                1: [wts_sb[:, 0:256]],
                2: [wts_sb[:, 256:512], wts_sb[:, 512:768]],
                3: [wts_sb[:, 768:784], wts_sb[:, 784:800]],
            }
            b_tiles = {1: bias_sb[:, 0:256], 2: bias_sb[:, 256:512],
                       3: bias_sb[:, 512:528]}

            def layer(l, gsrc, F_in, nF, F_out, h_big, last):
                for b, batch in enumerate(batches):
                    nb = len(batch)
                    msl = mpool.tile([P, nb * TL, F_in], dt.bfloat16,
                                     tag="msl", bufs=2, name=f"msl_{l}_{b}")
                    msh = mpool.tile([P, nb * TH, F_in], dt.bfloat16,
                                     tag="msh", bufs=2, name=f"msh_{l}_{b}")
                    c0 = batch[0]
                    # dma_gather caps at 1024 idxs/call -> chunk by 8 tiles
                    for off in range(0, nb * TL, 8):
                        ct = min(8, nb * TL - off)
                        nc.gpsimd.dma_gather(
                            out_ap=msl[:, off:off + ct, :],
                            in_ap=gsrc[0:LO_LIM, :],
                            idxs_ap=idxlo_sb[:, c0 * TL * 8 + off * 8:
                                             c0 * TL * 8 + (off + ct) * 8],
                            num_idxs=ct * P, num_idxs_reg=ct * P,
                            elem_size=F_in)
                    for off in range(0, nb * TH, 8):
                        ct = min(8, nb * TH - off)
                        nc.gpsimd.dma_gather(
                            out_ap=msh[:, off:off + ct, :],
                            in_ap=gsrc[HI_OFF:HI_OFF + LO_LIM, :],
                            idxs_ap=idxhi_sb[:, c0 * TH * 8 + off * 8:
                                             c0 * TH * 8 + (off + ct) * 8],
                            num_idxs=ct * P, num_idxs_reg=ct * P,
                            elem_size=F_in)
                    for bi, nt in enumerate(batch):
                        aggps = [pspool.tile([P, P], dt.float32, space="PSUM",
                                             tag=f"agg{fc}", bufs=2,
                                             name=f"agg_{l}_{nt}_{fc}")
                                 for fc in range(nF)]
                        for j in range(TNT):
                            g = nt * TNT + j
                            s_t = wpool.tile([P, P], dt.bfloat16, tag="s_t",
                                             bufs=4, name=f"s_{l}_{nt}_{j}")
                            nc.vector.tensor_scalar(
                                out=s_t[:], in0=iota_ap,
                                scalar1=meta_sb[:, dslot0 + g:dslot0 + g + 1],
                                scalar2=meta_sb[:, dinvd0 + g:dinvd0 + g + 1],
                                op0=Alu.is_equal, op1=Alu.mult)
                            if j < TL:
                                m_ap = msl[:, bi * TL + j, :]
                            else:
                                m_ap = msh[:, bi * TH + (j - TL), :]
                            for fc in range(nF):
                                nc.tensor.matmul(
                                    aggps[fc][:],
                                    lhsT=m_ap[:, fc * P:(fc + 1) * P],
                                    rhs=s_t[:],
                                    start=(j == 0), stop=(j == TNT - 1))
                        zps = pspool.tile([P, F_out], dt.float32, space="PSUM",
                                          tag="z", bufs=2, name=f"z_{l}_{nt}")
                        for fc in range(nF):
                            aggsb = wpool.tile([P, P], dt.bfloat16, tag="aggsb",
                                               bufs=3, name=f"aggsb_{l}_{nt}_{fc}")
                            nc.scalar.copy(out=aggsb[:], in_=aggps[fc][:])
                            nc.tensor.matmul(zps[:], lhsT=aggsb[:],
                                             rhs=w_chunks[l][fc],
                                             start=(fc == 0), stop=(fc == nF - 1))
                        tmp = wpool.tile([P, F_out], dt.float32, tag="tmp",
                                         bufs=3, name=f"tmp_{l}_{nt}")
                        nc.vector.tensor_tensor(out=tmp[:], in0=zps[:],
                                                in1=b_tiles[l], op=Alu.add)
                        if not last:
                            nc.scalar.activation(
                                out=h_big[:, nt, :], in_=tmp[:], func=Act.Relu,
                                scale=meta_sb[:, dinvn0 + nt:dinvn0 + nt + 1])
                        else:
                            mx = wpool.tile([P, 1], dt.float32, tag="mx",
                                            bufs=3, name=f"mx_{nt}")
                            nc.vector.tensor_reduce(
                                out=mx[:], in_=tmp[:, 0:F_OUT],
                                axis=mybir.AxisListType.X, op=Alu.max,
                                negate=True)
                            ex = wpool.tile([P, F_OUT], dt.float32, tag="ex",
                                            bufs=3, name=f"ex_{nt}")
                            nc.scalar.activation(out=ex[:], in_=tmp[:, 0:F_OUT],
                                                 func=Act.Exp, bias=mx[:])
                            sm = wpool.tile([P, 1], dt.float32, tag="sm",
                                            bufs=3, name=f"sm_{nt}")
                            nc.vector.tensor_reduce(
                                out=sm[:], in_=ex[:],
                                axis=mybir.AxisListType.X, op=Alu.add)
                            ls = wpool.tile([P, 1], dt.float32, tag="ls",
                                            bufs=3, name=f"ls_{nt}")
                            nc.scalar.activation(out=ls[:], in_=sm[:],
                                                 func=Act.Ln)
                            nls = wpool.tile([P, 1], dt.float32, tag="nls",
                                             bufs=3, name=f"nls_{nt}")
                            nc.vector.tensor_scalar(
                                out=nls[:], in0=ls[:], scalar1=-1.0,
                                scalar2=None, op0=Alu.mult)
                            nc.vector.tensor_scalar(
                                out=h_big[:, nt, 0:F_OUT], in0=tmp[:, 0:F_OUT],
                                scalar1=mx[:], scalar2=nls[:],
                                op0=Alu.add, op1=Alu.add)

            # Layer 1: gather x (128-wide)
            h1_big = bigpool.tile([P, NT, F_HID], dt.bfloat16)
            layer(1, x_pad, F_IN, 1, F_HID, h1_big, last=False)
            nc.sync.dma_start(
                out=h1_shard[:].rearrange("(t p) f -> p t f", p=P),
                in_=h1_big[:])
            nc.gpsimd.collective_compute(
                "AllGather", mybir.AluOpType.bypass,
                replica_groups=[list(range(NCORES))],
                ins=[h1_shard[:].opt()], outs=[h1_full[:].opt()])

            h2_big = bigpool.tile([P, NT, F_HID], dt.bfloat16)
            layer(2, h1_full, F_HID, 2, F_HID, h2_big, last=False)
            nc.sync.dma_start(
                out=h2_shard[:].rearrange("(t p) f -> p t f", p=P),
                in_=h2_big[:])
            nc.gpsimd.collective_compute(
                "AllGather", mybir.AluOpType.bypass,
                replica_groups=[list(range(NCORES))],
                ins=[h2_shard[:].opt()], outs=[h2_full[:].opt()])

            out_big = bigpool.tile([P, NT, F_OUT_P], dt.float32)
            nc.vector.memset(out_big[:], 0.0)
            layer(3, h2_full, F_HID, 2, F_OUT_P, out_big, last=True)
            nc.sync.dma_start(
                out=out_d[:].rearrange("(t p) f -> p t f", p=P),
                in_=out_big[:])

    nc.finalize()
    return nc


_CACHE = {}


def kernel(x, edge_index, W1, b1, W2, b2, W3, b3):
    from concourse.bass_utils import run_bass_kernel_spmd

    prep = _preprocess(x, edge_index)
    TL, TH, TNT = prep["TL"], prep["TH"], prep["TNT"]

    key = (TL, TH)
    if key not in _CACHE:
        _CACHE[key] = _build_program(TL, TH, TNT, prep["batches"])
    nc = _CACHE[key]

    W1 = np.asarray(W1, np.float32)
    W2 = np.asarray(W2, np.float32)
    W3 = np.asarray(W3, np.float32)
    wts = np.zeros((P, 256 + 512 + 32), dtype=BF16)
    wts[:, 0:256] = W1.astype(BF16)
    wts[:, 256:512] = W2[0:128].astype(BF16)
    wts[:, 512:768] = W2[128:256].astype(BF16)
    wts[:, 768:778] = W3[0:128].astype(BF16)
    wts[:, 784:794] = W3[128:256].astype(BF16)
    bias = np.zeros((P, 256 + 256 + 16), dtype=np.float32)
    bias[:, 0:256] = np.asarray(b1, np.float32)[None, :]
    bias[:, 256:512] = np.asarray(b2, np.float32)[None, :]
    bias[:, 512:522] = np.asarray(b3, np.float32)[None, :]

    in_maps = []
    for c in range(NCORES):
        m = dict(prep["cores"][c])
        m["x_pad"] = prep["x_pad"]
        m["wts"] = wts
        m["bias"] = bias
        in_maps.append(m)

    res = run_bass_kernel_spmd(nc, in_maps, core_ids=list(range(NCORES)))
    out = np.zeros((N, F_OUT), dtype=np.float32)
    for c in range(NCORES):
        out[c * NSH:(c + 1) * NSH] = res.results[c]["out"][:NSH, :F_OUT]
    return out



# revision 25
# speedup vs baseline: 1.4012x; 1.4012x over previous
"""3-layer GCN (CrystalGCN) on 8 TRN2 NeuronCores — hybrid pull/push.

Layer math (per layer): z = dinv_dst * (agg_raw @ W) + b, where
agg_raw[v] = sum_{u->v} g[u] and g = relu(z_prev) * dinv (src-side
prescale). Self-loop term g[v] handled separately in push stages.

Distribution:
  - L1 PULL: x is replicated; each core aggregates its own dst shard
    directly (one-hot S matmul scatter), gathering x~=x*dinv rows
    per-edge via gpsimd.dma_gather (lo/hi table split for int16 idx).
    Produces g1 (local shard) + g1T (transposed copy, on-chip).
  - L2 PUSH: each core processes edges whose src is local, gathers
    g1[src] from its local table, scatter-matmuls into per-window
    partials for ALL dst shards (f-major layout [8*256, cols]), then
    ONE ReduceScatter(add) per half -> agg2T own shard. No AllGather.
  - z2/y: z2T = W2^T @ (agg2T + g1T) in transposed layout; epilogue;
    y = g2 @ W3 (16 wide) per own tile.
  - L3 PUSH: same push tables; gathers y rows (256B), scatter-matmuls
    node-major partials [50176,16] fp32, ReduceScatter -> agg3;
    final epilogue + log_softmax fused, out.

SPMD: one program for all 8 cores; all per-window tile counts are
max-over-cores (data streams padded per core: gather idx 0, slot -1,
one-hot row becomes all-zero).
"""
import numpy as np
import ml_dtypes

N = 50000
E = 800000
F_IN, F_HID, F_OUT = 128, 256, 10
F_OUT_P = 16
NCORES = 8
NSH = N // NCORES            # 6250
P = 128
NT = (NSH + P - 1) // P      # 49
NSHP = NT * P                # 6272
NROWS = NSHP * NCORES        # 50176
HI_OFF = 17408
LO_LIM = 32768
WW = 256                     # window width
NW = 25                      # windows per shard: 24x256 + 1x128
ASPLIT = 16                  # push windows w<ASPLIT -> partial A
ACOLS = ASPLIT * WW          # 3072
BCOLS = NSHP - ACOLS         # 3200
BF16 = ml_dtypes.bfloat16

MAX_GROUP_TILES = 38         # gather-call group cap (tiles of 128 idxs)


def _wrap_idx16(vals):
    n = len(vals)
    assert n % 16 == 0
    blk = np.asarray(vals, dtype=np.int16).reshape(n // 16, 16).T
    return np.tile(blk, (8, 1))


def _win_width(w):
    return WW if w < NW - 1 else NSHP - (NW - 1) * WW  # 128 for w=24


def _preprocess(x, edge_index):
    x = np.asarray(x, dtype=np.float32)
    ei = np.asarray(edge_index, dtype=np.int64)
    loops = np.arange(N, dtype=np.int64)
    src_p = np.concatenate([ei[0], loops])   # pull streams include loops
    dst_p = np.concatenate([ei[1], loops])
    deg = np.bincount(dst_p, minlength=N).astype(np.float32)
    dinv = np.where(deg > 0, 1.0 / np.sqrt(deg), 0.0).astype(np.float32)

    x_pad = np.zeros((NROWS, F_IN), dtype=BF16)
    xs = (x * dinv[:, None]).astype(BF16)
    for c in range(NCORES):
        x_pad[c * NSHP:c * NSHP + NSH] = xs[c * NSH:(c + 1) * NSH]

    gidx_p = (src_p // NSH) * NSHP + (src_p % NSH)

    # ---------------- PULL (L1) ----------------
    c_of = dst_p // NSH
    loc = dst_p % NSH
    w_of = loc // WW                         # 0..24
    key = (c_of * NW + w_of)
    order = np.lexsort((gidx_p, key))
    g_s, loc_s, key_s = gidx_p[order], loc[order], key[order]
    starts = np.searchsorted(key_s, np.arange(NCORES * NW))
    ends = np.searchsorted(key_s, np.arange(NCORES * NW), side="right")

    pull_lo = {}
    pull_hi = {}
    for c in range(NCORES):
        for w in range(NW):
            k = c * NW + w
            g = g_s[starts[k]:ends[k]]
            sl = (loc_s[starts[k]:ends[k]] - w * WW).astype(np.float32)
            nlo = int(np.searchsorted(g, LO_LIM))
            pull_lo[(c, w)] = (g[:nlo], sl[:nlo])
            pull_hi[(c, w)] = (g[nlo:] - HI_OFF, sl[nlo:])
            assert nlo == len(g) or g[nlo:].min() >= HI_OFF

    klo = np.zeros(NW, np.int64)
    khi = np.zeros(NW, np.int64)
    for w in range(NW):
        klo[w] = max((len(pull_lo[(c, w)][0]) + P - 1) // P for c in range(NCORES))
        khi[w] = max((len(pull_hi[(c, w)][0]) + P - 1) // P for c in range(NCORES))
        klo[w] = max(klo[w], 1)
        khi[w] = max(khi[w], 1)

    # groups of 2 windows
    pull_groups = [list(range(i, min(i + 2, NW))) for i in range(0, NW, 2)]

    # ---------------- PUSH (L2/L3) ----------------
    src_l, dst_l = ei[0], ei[1]              # no self loops
    cs = src_l // NSH
    sloc = src_l % NSH
    dd = dst_l // NSH
    dloc = dst_l % NSH
    ww_of = dloc // WW
    # processing order: w-major (for A/B split), then dst core
    worder = [(w, d) for w in range(NW) for d in range(NCORES)]
    pos_of = {wd: i for i, wd in enumerate(worder)}
    pkey = np.array([pos_of[(w, d)] for w, d in zip(ww_of, dd)])
    porder = np.lexsort((sloc, cs * len(worder) + pkey))
    sloc_s = sloc[porder]
    slot_s = (dloc - ww_of * WW)[porder].astype(np.float32)
    pk_s = (cs * len(worder) + pkey)[porder]
    pstarts = np.searchsorted(pk_s, np.arange(NCORES * len(worder)))
    pends = np.searchsorted(pk_s, np.arange(NCORES * len(worder)), side="right")

    k2 = np.zeros(len(worder), np.int64)
    for i in range(len(worder)):
        k2[i] = max(pends[c * len(worder) + i] - pstarts[c * len(worder) + i]
                    for c in range(NCORES))
        k2[i] = max((k2[i] + P - 1) // P, 1)

    # push gather groups: the 8 dst-core windows of one w block
    push_groups = [list(range(w * NCORES, (w + 1) * NCORES)) for w in range(NW)]

    sig = (tuple(klo), tuple(khi), tuple(k2))

    # ---------------- per-core data streams ----------------
    npt = int(klo.sum() + khi.sum())
    npp = int(k2.sum())
    cores = []
    for c in range(NCORES):
        lo_stream = []
        hi_stream = []
        slot_cols = np.full((P, npt), -1.0, np.float32)
        scol = 0
        for grp in pull_groups:
            # buffer order: lo tiles of each window, then hi tiles
            for w in grp:
                g, sl = pull_lo[(c, w)]
                n = klo[w] * P
                a = np.zeros(n, np.int64)
                a[:len(g)] = g
                lo_stream.append(a)
                m = np.arange(len(g))
                sc = np.full((P, klo[w]), -1.0, np.float32)
                sc[m % P, m // P] = sl
                slot_cols[:, scol:scol + klo[w]] = sc
                scol += klo[w]
            for w in grp:
                g, sl = pull_hi[(c, w)]
                n = khi[w] * P
                a = np.zeros(n, np.int64)
                a[:len(g)] = g
                hi_stream.append(a)
                m = np.arange(len(g))
                sc = np.full((P, khi[w]), -1.0, np.float32)
                sc[m % P, m // P] = sl
                slot_cols[:, scol:scol + khi[w]] = sc
                scol += khi[w]
        assert scol == npt
        idx_lo = _wrap_idx16(np.concatenate(lo_stream))
        idx_hi = _wrap_idx16(np.concatenate(hi_stream))

        push_stream = []
        pslot_cols = np.full((P, npp), -1.0, np.float32)
        scol = 0
        for i, (w, d) in enumerate(worder):
            k = c * len(worder) + i
            g = sloc_s[pstarts[k]:pends[k]]
            sl = slot_s[pstarts[k]:pends[k]]
            n = k2[i] * P
            a = np.zeros(n, np.int64)
            a[:len(g)] = g
            push_stream.append(a)
            m = np.arange(len(g))
            sc = np.full((P, k2[i]), -1.0, np.float32)
            sc[m % P, m // P] = sl
            pslot_cols[:, scol:scol + k2[i]] = sc
            scol += k2[i]
        assert scol == npp
        idx_push = _wrap_idx16(np.concatenate(push_stream))

        dinvd = np.zeros((P, NT), np.float32)
        lm = np.arange(NSH)
        dinvd[lm % P, lm // P] = dinv[c * NSH:(c + 1) * NSH]
        dinvbc = np.zeros(NSHP, np.float32)
        dinvbc[:NSH] = dinv[c * NSH:(c + 1) * NSH]
        dinvbc = np.broadcast_to(dinvbc.astype(BF16), (P, NSHP))

        cores.append({
            "idx_lo": idx_lo, "idx_hi": idx_hi, "idx_push": idx_push,
            "slot_pull": slot_cols, "slot_push": pslot_cols,
            "dinvd": dinvd, "dinvbc": np.ascontiguousarray(dinvbc),
        })

    layout = {
        "klo": klo, "khi": khi, "k2": k2,
        "pull_groups": pull_groups, "push_groups": push_groups,
        "worder": worder, "npt": npt, "npp": npp,
    }
    return {"x_pad": x_pad, "cores": cores, "layout": layout, "sig": sig}


# ---------------- meta/weights packing ----------------
# meta fp32 [128, npt + npp + NT + 2 + 256 + 16]
# wts bf16 [128, 256 W1 | 512 W2 | 32 W3 | 256 iota | 128 id | 6272 dinvbc]
def _pack_meta(core, W_np):
    npt = core["slot_pull"].shape[1]
    npp = core["slot_push"].shape[1]
    b1, b2, b3 = W_np["b1"], W_np["b2"], W_np["b3"]
    M = npt + npp + NT + 2 + 256 + 2 * NT * F_OUT_P
    meta = np.zeros((P, M), np.float32)
    o = 0
    meta[:, o:o + npt] = core["slot_pull"]; o += npt
    meta[:, o:o + npp] = core["slot_push"]; o += npp
    meta[:, o:o + NT] = core["dinvd"]; o += NT
    meta[:, o] = b2[np.arange(P)]
    meta[:, o + 1] = b2[128 + np.arange(P)]
    o += 2
    meta[:, o:o + 256] = b1[None, :]; o += 256
    # dinvd3: dinv[dst] repeated over the 16 output cols, per tile
    meta[:, o:o + NT * F_OUT_P] = np.repeat(
        core["dinvd"], F_OUT_P, axis=1).reshape(P, NT * F_OUT_P)
    o += NT * F_OUT_P
    b3r = np.zeros((P, NT * F_OUT_P), np.float32)
    b3r.reshape(P, NT, F_OUT_P)[:, :, 0:F_OUT] = b3[None, None, :]
    meta[:, o:o + NT * F_OUT_P] = b3r
    return meta


def _pack_wts(core, W_np):
    W1, W2, W3 = W_np["W1"], W_np["W2"], W_np["W3"]
    wts = np.zeros((P, 256 + 512 + 32 + 256 + 128 + NSHP), dtype=BF16)
    wts[:, 0:256] = W1.astype(BF16)
    wts[:, 256:512] = W2[0:128].astype(BF16)
    wts[:, 512:768] = W2[128:256].astype(BF16)
    wts[:, 768:784] = W3[0:128, :].astype(BF16)
    wts[:, 784:800] = W3[128:256, :].astype(BF16)
    wts[:, 800:1056] = np.arange(256, dtype=np.float32).astype(BF16)[None, :]
    wts[:, 1056:1184] = np.eye(128, dtype=np.float32).astype(BF16)
    wts[:, 1184:1184 + NSHP] = core["dinvbc"]
    return wts


def _build_program(layout):
    import concourse.bass as bass
    from concourse import bacc
    import concourse.mybir as mybir
    from concourse.tile import TileContext

    dt = mybir.dt
    Alu = mybir.AluOpType
    Act = mybir.ActivationFunctionType

    klo, khi, k2 = layout["klo"], layout["khi"], layout["k2"]
    pull_groups, push_groups = layout["pull_groups"], layout["push_groups"]
    worder = layout["worder"]
    npt, npp = layout["npt"], layout["npp"]
    M = npt + npp + NT + 2 + 256 + 2 * NT * F_OUT_P
    WCOLS = 256 + 512 + 32 + 256 + 128 + NSHP

    nc = bacc.Bacc(num_devices=NCORES)
    x_pad = nc.dram_tensor("x_pad", [NROWS, F_IN], dt.bfloat16, kind="ExternalInput")
    idx_lo_d = nc.dram_tensor("idx_lo", [P, int(klo.sum()) * 8], dt.int16, kind="ExternalInput")
    idx_hi_d = nc.dram_tensor("idx_hi", [P, int(khi.sum()) * 8], dt.int16, kind="ExternalInput")
    idx_push_d = nc.dram_tensor("idx_push", [P, int(k2.sum()) * 8], dt.int16, kind="ExternalInput")
    meta_d = nc.dram_tensor("meta", [P, M], dt.float32, kind="ExternalInput")
    wts_d = nc.dram_tensor("wts", [P, WCOLS], dt.bfloat16, kind="ExternalInput")
    out_d = nc.dram_tensor("out", [NSHP, F_OUT_P], dt.float32, kind="ExternalOutput")

    with TileContext(nc) as tc:
        with tc.tile_pool(name="const", bufs=1) as cpool, \
             tc.tile_pool(name="msgs", bufs=2) as mpool, \
             tc.tile_pool(name="work", bufs=3) as wpool, \
             tc.tile_pool(name="spool", bufs=1) as spool, \
             tc.tile_pool(name="persist", bufs=1) as ppool, \
             tc.tile_pool(name="ps", bufs=2, space="PSUM") as pspool, \
             tc.tile_pool(name="dram", bufs=1, space="DRAM") as dpool:

            idxlo_sb = cpool.tile([P, int(klo.sum()) * 8], dt.int16)
            nc.sync.dma_start(out=idxlo_sb[:], in_=idx_lo_d[:])
            idxhi_sb = cpool.tile([P, int(khi.sum()) * 8], dt.int16)
            nc.sync.dma_start(out=idxhi_sb[:], in_=idx_hi_d[:])
            idxp_sb = cpool.tile([P, int(k2.sum()) * 8], dt.int16)
            nc.sync.dma_start(out=idxp_sb[:], in_=idx_push_d[:])
            meta_sb = cpool.tile([P, M], dt.float32)
            nc.sync.dma_start(out=meta_sb[:], in_=meta_d[:])
            wts_sb = cpool.tile([P, WCOLS], dt.bfloat16)
            nc.sync.dma_start(out=wts_sb[:], in_=wts_d[:])

            slot_pull = meta_sb[:, 0:npt]
            slot_push = meta_sb[:, npt:npt + npp]
            dinvd = meta_sb[:, npt + npp:npt + npp + NT]
            b2c = meta_sb[:, npt + npp + NT:npt + npp + NT + 2]
            b1bc = meta_sb[:, npt + npp + NT + 2:npt + npp + NT + 2 + 256]
            _o3 = npt + npp + NT + 2 + 256
            dinvd3 = meta_sb[:, _o3:_o3 + NT * F_OUT_P]
            b3rep = meta_sb[:, _o3 + NT * F_OUT_P:M]
            w1_sb = wts_sb[:, 0:256]
            w2_sb = [wts_sb[:, 256:512], wts_sb[:, 512:768]]
            w3_sb = [wts_sb[:, 768:784], wts_sb[:, 784:800]]
            iota = wts_sb[:, 800:1056]
            ident = wts_sb[:, 1056:1184]
            dinvbc = wts_sb[:, 1184:1184 + NSHP]

            g1_d = dpool.tile([NSHP, F_HID], dt.bfloat16)
            y_d = dpool.tile([NSHP, 128], dt.bfloat16)
            pA = dpool.tile([NCORES * 256, ACOLS], dt.bfloat16)
            pB = dpool.tile([NCORES * 256, BCOLS], dt.bfloat16)
            aggA = dpool.tile([256, ACOLS], dt.bfloat16)
            aggB = dpool.tile([256, BCOLS], dt.bfloat16)
            p3_d = dpool.tile([NROWS, F_OUT_P], dt.float32)
            agg3_d = dpool.tile([NSHP, F_OUT_P], dt.float32)

            g1T_sb = ppool.tile([P, 2, NSHP], dt.bfloat16)
            ysb = ppool.tile([P, NT, F_OUT_P], dt.bfloat16)

            # ================= L1 PULL =================
            lo_off = 0
            hi_off = 0
            scol = 0
            for grp in pull_groups:
                nlo_g = int(sum(klo[w] for w in grp))
                nhi_g = int(sum(khi[w] for w in grp))
                ntg = nlo_g + nhi_g
                msl = mpool.tile([P, ntg, F_IN], dt.bfloat16, tag="msl",
                                 bufs=2, name=f"msl1_{grp[0]}")
                for off in range(0, nlo_g, 8):
                    ct = min(8, nlo_g - off)
                    nc.gpsimd.dma_gather(
                        out_ap=msl[:, off:off + ct, :], in_ap=x_pad[0:LO_LIM, :],
                        idxs_ap=idxlo_sb[:, (lo_off + off) * 8:
                                         (lo_off + off + ct) * 8],
                        num_idxs=ct * P, num_idxs_reg=ct * P,
                        elem_size=F_IN)
                for off in range(0, nhi_g, 8):
                    ct = min(8, nhi_g - off)
                    nc.gpsimd.dma_gather(
                        out_ap=msl[:, nlo_g + off:nlo_g + off + ct, :],
                        in_ap=x_pad[HI_OFF:HI_OFF + LO_LIM, :],
                        idxs_ap=idxhi_sb[:, (hi_off + off) * 8:
                                         (hi_off + off + ct) * 8],
                        num_idxs=ct * P, num_idxs_reg=ct * P,
                        elem_size=F_IN)
                g1stage = wpool.tile([P, 4, 256], dt.bfloat16, tag="g1stage",
                                     bufs=2, name=f"g1stage_{grp[0]}")
                # per-window buffer positions
                lo_rel = 0
                hi_rel = nlo_g
                sc_lo = scol
                sc_hi = scol + nlo_g
                for w in grp:
                    ww = _win_width(w)
                    agg = pspool.tile([P, ww], dt.float32, space="PSUM",
                                      tag="psA", bufs=3, name=f"agg1_{w}")
                    tiles = [(lo_rel + j, sc_lo + j) for j in range(int(klo[w]))] + \
                            [(hi_rel + j, sc_hi + j) for j in range(int(khi[w]))]
                    for jj, (bufpos, sc) in enumerate(tiles):
                        s_t = spool.tile([P, 256], dt.bfloat16, tag="s",
                                         bufs=16, name=f"s1_{w}_{jj}")
                        nc.vector.tensor_scalar(
                            out=s_t[:, 0:ww], in0=iota[:, 0:ww],
                            scalar1=slot_pull[:, sc:sc + 1], scalar2=None,
                            op0=Alu.is_equal)
                        nc.tensor.matmul(
                            agg[:], lhsT=msl[:, bufpos, :], rhs=s_t[:, 0:ww],
                            start=(jj == 0), stop=(jj == len(tiles) - 1))
                    lo_rel += int(klo[w]); sc_lo += int(klo[w])
                    hi_rel += int(khi[w]); sc_hi += int(khi[w])
                    aggsb = wpool.tile([P, ww], dt.bfloat16, tag="aggsb",
                                       bufs=3, name=f"aggsb_{w}")
                    nc.scalar.copy(out=aggsb[:], in_=agg[:])
                    nst = ww // P
                    for st in range(nst):
                        t_idx = w * 2 + st
                        zps = pspool.tile([P, 256], dt.float32, space="PSUM",
                                          tag="psB", bufs=3, name=f"z1_{t_idx}")
                        nc.tensor.matmul(zps[:], lhsT=aggsb[:, st * P:(st + 1) * P],
                                         rhs=w1_sb, start=True, stop=True)
                        t1 = wpool.tile([P, 256], dt.float32, tag="ep1",
                                        bufs=3, name=f"t1_{t_idx}")
                        nc.vector.tensor_scalar(
                            out=t1[:], in0=zps[:],
                            scalar1=dinvd[:, t_idx:t_idx + 1], scalar2=None,
                            op0=Alu.mult)
                        t2 = wpool.tile([P, 256], dt.float32, tag="ep2",
                                        bufs=3, name=f"t2_{t_idx}")
                        nc.vector.tensor_tensor(out=t2[:], in0=t1[:], in1=b1bc,
                                                op=Alu.add)
                        g1t = g1stage[:, t_idx - grp[0] * 2, :]
                        nc.scalar.activation(out=g1t, in_=t2[:], func=Act.Relu,
                                             scale=dinvd[:, t_idx:t_idx + 1])
                        for fc in range(2):
                            tp = pspool.tile([P, P], dt.bfloat16, space="PSUM",
                                             tag="psC", bufs=2,
                                             name=f"tp_{t_idx}_{fc}")
                            nc.tensor.transpose(tp[:], g1t[:, fc * P:(fc + 1) * P],
                                                ident)
                            nc.scalar.copy(
                                out=g1T_sb[:, fc, t_idx * P:(t_idx + 1) * P],
                                in_=tp[:])
                t0 = grp[0] * 2
                nst_g = sum(_win_width(w) // P for w in grp)
                nc.sync.dma_start(
                    out=g1_d[t0 * P:(t0 + nst_g) * P, :].rearrange(
                        "(t p) f -> p t f", p=P),
                    in_=g1stage[:, 0:nst_g, :])
                scol += ntg
                lo_off += nlo_g
                hi_off += nhi_g

            # ================= L2 PUSH =================
            k2pre = [0]
            for i in range(len(worder)):
                k2pre.append(k2pre[-1] + int(k2[i]))

            def push_stage(stage, gsrc, felem, w_lo, w_hi, rs_hook=None):
                """stage 2|3 over w blocks [w_lo, w_hi); one partial DMA per w."""
                for w in range(w_lo, w_hi):
                    ww = _win_width(w)
                    if stage == 2:
                        pstg = wpool.tile([P, 16, WW], dt.bfloat16, tag="p2st",
                                          bufs=2, name=f"p2st_{w}")
                    else:
                        pstg = wpool.tile([P, NCORES, 2, F_OUT_P], dt.float32,
                                          bufs=2, tag="p3st", name=f"p3st_{w}")
                    grp = push_groups[w]
                    p_off = k2pre[grp[0]]
                    ntg = k2pre[grp[-1] + 1] - p_off
                    msl = mpool.tile([P, ntg, felem], dt.bfloat16, tag="msl",
                                     bufs=2, name=f"msl{stage}_{grp[0]}")
                    for off in range(0, ntg, 8):
                        ct = min(8, ntg - off)
                        nc.gpsimd.dma_gather(
                            out_ap=msl[:, off:off + ct, :], in_ap=gsrc,
                            idxs_ap=idxp_sb[:, (p_off + off) * 8:
                                            (p_off + off + ct) * 8],
                            num_idxs=ct * P, num_idxs_reg=ct * P,
                            elem_size=felem)
                    if rs_hook is not None and w == rs_hook[0]:
                        rs_hook[1]()
                    rel = 0
                    for i in grp:
                        _, dcore = worder[i]
                        sc = k2pre[i]
                        ktiles = int(k2[i])
                        if stage == 2:
                            pps = [pspool.tile([P, ww], dt.float32, space="PSUM",
                                               tag=f"ps{chr(65+fc)}", bufs=3,
                                               name=f"p2_{i}_{fc}")
                                   for fc in range(2)]
                        else:
                            nh = ww // P
                            pps = [pspool.tile([P, F_OUT_P], dt.float32,
                                               space="PSUM",
                                               tag=f"ps{chr(65+h)}", bufs=3,
                                               name=f"p3_{i}_{h}")
                                   for h in range(nh)]
                        for jj in range(ktiles):
                            s_t = spool.tile([P, 256], dt.bfloat16, tag="s",
                                             bufs=16, name=f"s{stage}_{i}_{jj}")
                            nc.vector.tensor_scalar(
                                out=s_t[:, 0:ww], in0=iota[:, 0:ww],
                                scalar1=slot_push[:, sc + jj:sc + jj + 1],
                                scalar2=None, op0=Alu.is_equal)
                            first, last = (jj == 0), (jj == ktiles - 1)
                            if stage == 2:
                                for fc in range(2):
                                    nc.tensor.matmul(
                                        pps[fc][:],
                                        lhsT=msl[:, rel + jj,
                                                 fc * P:(fc + 1) * P],
                                        rhs=s_t[:, 0:ww],
                                        start=first, stop=last)
                            else:
                                for h in range(len(pps)):
                                    nc.tensor.matmul(
                                        pps[h][:],
                                        lhsT=s_t[:, h * P:(h + 1) * P],
                                        rhs=msl[:, rel + jj, 0:F_OUT_P],
                                        start=first, stop=last)
                        if stage == 2:
                            for fc in range(2):
                                nc.scalar.copy(
                                    out=pstg[:, dcore * 2 + fc, 0:ww],
                                    in_=pps[fc][:])
                        else:
                            for h in range(len(pps)):
                                nc.scalar.copy(out=pstg[:, dcore, h, :],
                                               in_=pps[h][:])
                        rel += ktiles
                    if stage == 2:
                        if w < ASPLIT:
                            tgt, col0 = pA, w * WW
                        else:
                            tgt, col0 = pB, (w - ASPLIT) * WW
                        nc.sync.dma_start(
                            out=tgt[:, col0:col0 + ww].rearrange(
                                "(a p) f -> p a f", p=P),
                            in_=pstg[:, :, 0:ww])
                    else:
                        nh = ww // P
                        for h in range(nh):
                            nc.sync.dma_start(
                                out=p3_d[:].rearrange(
                                    "(d r p) f -> p d r f", d=NCORES, p=P)[
                                    :, :, 2 * w + h, :],
                                in_=pstg[:, :, h, :])

            def _rs_a():
                nc.gpsimd.collective_compute(
                    "ReduceScatter", mybir.AluOpType.add,
                    replica_groups=[list(range(NCORES))],
                    ins=[pA[:].opt()], outs=[aggA[:].opt()])

            push_stage(2, g1_d[:], F_HID, 0, ASPLIT)
            push_stage(2, g1_d[:], F_HID, ASPLIT, NW, rs_hook=(19, _rs_a))

            # ================= z2 / y =================
            chunks = [(i * 512, 512) for i in range(12)] + [(6144, P)]

            def z2_chunk(c0, cw):
                a2 = wpool.tile([P, 2, 512], dt.bfloat16, tag="a2", bufs=2,
                                name=f"a2_{c0}")
                if c0 < ACOLS:
                    src = aggA[:, c0:c0 + cw]
                else:
                    src = aggB[:, c0 - ACOLS:c0 - ACOLS + cw]
                nc.sync.dma_start(
                    out=a2[:, :, 0:cw],
                    in_=src.rearrange("(a p) d -> p a d", p=P))
                g2 = wpool.tile([P, 2, 512], dt.bfloat16, tag="g2", bufs=2,
                                name=f"g2_{c0}")
                for fo in range(2):
                    zps = pspool.tile([P, 512], dt.float32, space="PSUM",
                                      tag="psA", bufs=3, name=f"z2_{c0}_{fo}")
                    for fi in range(2):
                        nc.tensor.matmul(
                            zps[:, 0:cw], lhsT=w2_sb[fi][:, fo * P:(fo + 1) * P],
                            rhs=a2[:, fi, 0:cw], start=(fi == 0), stop=False)
                    for fi in range(2):
                        nc.tensor.matmul(
                            zps[:, 0:cw], lhsT=w2_sb[fi][:, fo * P:(fo + 1) * P],
                            rhs=g1T_sb[:, fi, c0:c0 + cw],
                            start=False, stop=(fi == 1))
                    t1 = wpool.tile([P, 512], dt.float32, tag="zt1", bufs=2,
                                    name=f"zt1_{c0}_{fo}")
                    nc.vector.tensor_tensor(out=t1[:, 0:cw], in0=zps[:, 0:cw],
                                            in1=dinvbc[:, c0:c0 + cw],
                                            op=Alu.mult)
                    h2 = wpool.tile([P, 512], dt.float32, tag="zh", bufs=2,
                                    name=f"zh_{c0}_{fo}")
                    nc.vector.tensor_scalar(
                        out=h2[:, 0:cw], in0=t1[:, 0:cw],
                        scalar1=b2c[:, fo:fo + 1], scalar2=0.0,
                        op0=Alu.add, op1=Alu.max)
                    nc.vector.tensor_tensor(out=g2[:, fo, 0:cw], in0=h2[:, 0:cw],
                                            in1=dinvbc[:, c0:c0 + cw],
                                            op=Alu.mult)
                for t_idx in range(c0 // P, (c0 + cw) // P):
                    off = t_idx * P - c0
                    yps = pspool.tile([P, F_OUT_P], dt.float32, space="PSUM",
                                      tag="psB", bufs=3, name=f"y_{t_idx}")
                    for fi in range(2):
                        nc.tensor.matmul(yps[:],
                                         lhsT=g2[:, fi, off:off + P],
                                         rhs=w3_sb[fi],
                                         start=(fi == 0), stop=(fi == 1))
                    nc.vector.tensor_copy(out=ysb[:, t_idx, :], in_=yps[:])

            for (c0, cw) in chunks[:8]:
                z2_chunk(c0, cw)
            nc.gpsimd.collective_compute(
                "ReduceScatter", mybir.AluOpType.add,
                replica_groups=[list(range(NCORES))],
                ins=[pB[:].opt()], outs=[aggB[:].opt()])
            for (c0, cw) in chunks[8:]:
                z2_chunk(c0, cw)
            nc.sync.dma_start(
                out=y_d[0:ACOLS, 0:F_OUT_P].rearrange("(t p) f -> p t f", p=P),
                in_=ysb[:, 0:ACOLS // P, :])
            nc.sync.dma_start(
                out=y_d[ACOLS:NSHP, 0:F_OUT_P].rearrange(
                    "(t p) f -> p t f", p=P),
                in_=ysb[:, ACOLS // P:NT, :])

            # ================= L3 PUSH =================
            push_stage(3, y_d[:], 128, 0, NW)

            nc.gpsimd.collective_compute(
                "ReduceScatter", mybir.AluOpType.add,
                replica_groups=[list(range(NCORES))],
                ins=[p3_d[:].opt()], outs=[agg3_d[:].opt()])

            # ================= final epilogue (batched) =================
            agg3sb = ppool.tile([P, NT, F_OUT_P], dt.float32)
            nc.sync.dma_start(
                out=agg3sb[:],
                in_=agg3_d[:].rearrange("(t p) f -> p t f", p=P))
            outsb = ppool.tile([P, NT, F_OUT_P], dt.float32)
            a1 = wpool.tile([P, NT, F_OUT_P], dt.float32, tag="ftmp", bufs=2,
                            name="fa1")
            nc.vector.tensor_tensor(out=a1[:], in0=agg3sb[:], in1=ysb[:],
                                    op=Alu.add)
            t2b = wpool.tile([P, NT, F_OUT_P], dt.float32, tag="ftmp", bufs=2,
                             name="ft2b")
            nc.vector.tensor_tensor(
                out=t2b[:], in0=a1[:],
                in1=dinvd3.rearrange("p (t f) -> p t f", f=F_OUT_P),
                op=Alu.mult)
            t3b = ppool.tile([P, NT, F_OUT_P], dt.float32)
            nc.vector.tensor_tensor(
                out=t3b[:], in0=t2b[:],
                in1=b3rep.rearrange("p (t f) -> p t f", f=F_OUT_P),
                op=Alu.add)
            ex_all = ppool.tile([P, NT, F_OUT], dt.float32)
            nc.scalar.activation(out=ex_all[:], in_=t3b[:, :, 0:F_OUT],
                                 func=Act.Exp)
            sm_all = ppool.tile([P, NT], dt.float32)
            for t_idx in range(NT):
                nc.vector.tensor_reduce(
                    out=sm_all[:, t_idx:t_idx + 1], in_=ex_all[:, t_idx, :],
                    axis=mybir.AxisListType.X, op=Alu.add)
            ls_all = ppool.tile([P, NT], dt.float32)
            nc.scalar.activation(out=ls_all[:], in_=sm_all[:], func=Act.Ln)
            for t_idx in range(NT):
                nc.vector.tensor_scalar(
                    out=outsb[:, t_idx, 0:F_OUT], in0=t3b[:, t_idx, 0:F_OUT],
                    scalar1=ls_all[:, t_idx:t_idx + 1], scalar2=None,
                    op0=Alu.subtract)
            nc.sync.dma_start(
                out=out_d[:].rearrange("(t p) f -> p t f", p=P),
                in_=outsb[:])

    nc.finalize()
    return nc


_CACHE = {}


def kernel(x, edge_index, W1, b1, W2, b2, W3, b3):
    from concourse.bass_utils import run_bass_kernel_spmd

    prep = _preprocess(x, edge_index)
    layout = prep["layout"]

    if prep["sig"] not in _CACHE:
        _CACHE[prep["sig"]] = _build_program(layout)
    nc = _CACHE[prep["sig"]]

    W_np = {
        "W1": np.asarray(W1, np.float32), "b1": np.asarray(b1, np.float32),
        "W2": np.asarray(W2, np.float32), "b2": np.asarray(b2, np.float32),
        "W3": np.zeros((256, F_OUT_P), np.float32),
        "b3": np.asarray(b3, np.float32),
    }
    W_np["W3"][:, 0:F_OUT] = np.asarray(W3, np.float32)

    in_maps = []
    for c in range(NCORES):
        core = prep["cores"][c]
        in_maps.append({
            "x_pad": prep["x_pad"],
            "idx_lo": core["idx_lo"], "idx_hi": core["idx_hi"],
            "idx_push": core["idx_push"],
            "meta": _pack_meta(core, W_np),
            "wts": _pack_wts(core, W_np),
        })

    res = run_bass_kernel_spmd(nc, in_maps, core_ids=list(range(NCORES)))
    out = np.zeros((N, F_OUT), dtype=np.float32)
    for c in range(NCORES):
        out[c * NSH:(c + 1) * NSH] = res.results[c]["out"][:NSH, :F_OUT]
    return out


# revision 30
# speedup vs baseline: 1.4038x; 1.0019x over previous
"""3-layer GCN (CrystalGCN) on 8 TRN2 NeuronCores — hybrid pull/push.

Layer math (per layer): z = dinv_dst * (agg_raw @ W) + b, where
agg_raw[v] = sum_{u->v} g[u] and g = relu(z_prev) * dinv (src-side
prescale). Self-loop term g[v] handled separately in push stages.

Distribution:
  - L1 PULL: x is replicated; each core aggregates its own dst shard
    directly (one-hot S matmul scatter), gathering x~=x*dinv rows
    per-edge via gpsimd.dma_gather (lo/hi table split for int16 idx).
    Produces g1 (local shard) + g1T (transposed copy, on-chip).
  - L2 PUSH: each core processes edges whose src is local, gathers
    g1[src] from its local table, scatter-matmuls into per-window
    partials for ALL dst shards (f-major layout [8*256, cols]), then
    ONE ReduceScatter(add) per half -> agg2T own shard. No AllGather.
  - z2/y: z2T = W2^T @ (agg2T + g1T) in transposed layout; epilogue;
    y = g2 @ W3 (16 wide) per own tile.
  - L3 PUSH: same push tables; gathers y rows (256B), scatter-matmuls
    node-major partials [50176,16] fp32, ReduceScatter -> agg3;
    final epilogue + log_softmax fused, out.

SPMD: one program for all 8 cores; all per-window tile counts are
max-over-cores (data streams padded per core: gather idx 0, slot -1,
one-hot row becomes all-zero).
"""
import numpy as np
import ml_dtypes

N = 50000
E = 800000
F_IN, F_HID, F_OUT = 128, 256, 10
F_OUT_P = 16
NCORES = 8
NSH = N // NCORES            # 6250
P = 128
NT = (NSH + P - 1) // P      # 49
NSHP = NT * P                # 6272
NROWS = NSHP * NCORES        # 50176
HI_OFF = 17408
LO_LIM = 32768
WW = 256                     # window width
NW = 25                      # windows per shard: 24x256 + 1x128
ASPLIT = 16                  # push windows w<ASPLIT -> partial A
ACOLS = ASPLIT * WW          # 3072
BCOLS = NSHP - ACOLS         # 3200
BF16 = ml_dtypes.bfloat16

MAX_GROUP_TILES = 38         # gather-call group cap (tiles of 128 idxs)


def _wrap_idx16(vals):
    n = len(vals)
    assert n % 16 == 0
    blk = np.asarray(vals, dtype=np.int16).reshape(n // 16, 16).T
    return np.tile(blk, (8, 1))


def _win_width(w):
    return WW if w < NW - 1 else NSHP - (NW - 1) * WW  # 128 for w=24


def _preprocess(x, edge_index):
    x = np.asarray(x, dtype=np.float32)
    ei = np.asarray(edge_index, dtype=np.int64)
    loops = np.arange(N, dtype=np.int64)
    src_p = np.concatenate([ei[0], loops])   # pull streams include loops
    dst_p = np.concatenate([ei[1], loops])
    deg = np.bincount(dst_p, minlength=N).astype(np.float32)
    dinv = np.where(deg > 0, 1.0 / np.sqrt(deg), 0.0).astype(np.float32)

    x_pad = np.zeros((NROWS, F_IN), dtype=BF16)
    xs = (x * dinv[:, None]).astype(BF16)
    for c in range(NCORES):
        x_pad[c * NSHP:c * NSHP + NSH] = xs[c * NSH:(c + 1) * NSH]

    gidx_p = (src_p // NSH) * NSHP + (src_p % NSH)

    # ---------------- PULL (L1) ----------------
    c_of = dst_p // NSH
    loc = dst_p % NSH
    w_of = loc // WW                         # 0..24
    key = (c_of * NW + w_of)
    order = np.lexsort((gidx_p, key))
    g_s, loc_s, key_s = gidx_p[order], loc[order], key[order]
    starts = np.searchsorted(key_s, np.arange(NCORES * NW))
    ends = np.searchsorted(key_s, np.arange(NCORES * NW), side="right")

    pull_lo = {}
    pull_hi = {}
    for c in range(NCORES):
        for w in range(NW):
            k = c * NW + w
            g = g_s[starts[k]:ends[k]]
            sl = (loc_s[starts[k]:ends[k]] - w * WW).astype(np.float32)
            nlo = int(np.searchsorted(g, LO_LIM))
            pull_lo[(c, w)] = (g[:nlo], sl[:nlo])
            pull_hi[(c, w)] = (g[nlo:] - HI_OFF, sl[nlo:])
            assert nlo == len(g) or g[nlo:].min() >= HI_OFF

    klo = np.zeros(NW, np.int64)
    khi = np.zeros(NW, np.int64)
    for w in range(NW):
        klo[w] = max((len(pull_lo[(c, w)][0]) + P - 1) // P for c in range(NCORES))
        khi[w] = max((len(pull_hi[(c, w)][0]) + P - 1) // P for c in range(NCORES))
        klo[w] = max(klo[w], 1)
        khi[w] = max(khi[w], 1)

    # groups of 2 windows
    pull_groups = [list(range(i, min(i + 2, NW))) for i in range(0, NW, 2)]

    # ---------------- PUSH (L2/L3) ----------------
    src_l, dst_l = ei[0], ei[1]              # no self loops
    cs = src_l // NSH
    sloc = src_l % NSH
    dd = dst_l // NSH
    dloc = dst_l % NSH
    ww_of = dloc // WW
    # processing order: w-major (for A/B split), then dst core
    worder = [(w, d) for w in range(NW) for d in range(NCORES)]
    pos_of = {wd: i for i, wd in enumerate(worder)}
    pkey = np.array([pos_of[(w, d)] for w, d in zip(ww_of, dd)])
    porder = np.lexsort((sloc, cs * len(worder) + pkey))
    sloc_s = sloc[porder]
    slot_s = (dloc - ww_of * WW)[porder].astype(np.float32)
    pk_s = (cs * len(worder) + pkey)[porder]
    pstarts = np.searchsorted(pk_s, np.arange(NCORES * len(worder)))
    pends = np.searchsorted(pk_s, np.arange(NCORES * len(worder)), side="right")

    k2 = np.zeros(len(worder), np.int64)
    for i in range(len(worder)):
        k2[i] = max(pends[c * len(worder) + i] - pstarts[c * len(worder) + i]
                    for c in range(NCORES))
        k2[i] = max((k2[i] + P - 1) // P, 1)

    # push gather groups: the 8 dst-core windows of one w block
    push_groups = [list(range(w * NCORES, (w + 1) * NCORES)) for w in range(NW)]

    sig = (tuple(klo), tuple(khi), tuple(k2))

    # ---------------- per-core data streams ----------------
    npt = int(klo.sum() + khi.sum())
    npp = int(k2.sum())
    cores = []
    for c in range(NCORES):
        lo_stream = []
        hi_stream = []
        slot_cols = np.full((P, npt), -1.0, np.float32)
        scol = 0
        for grp in pull_groups:
            # buffer order: lo tiles of each window, then hi tiles
            for w in grp:
                g, sl = pull_lo[(c, w)]
                n = klo[w] * P
                a = np.zeros(n, np.int64)
                a[:len(g)] = g
                lo_stream.append(a)
                m = np.arange(len(g))
                sc = np.full((P, klo[w]), -1.0, np.float32)
                sc[m % P, m // P] = sl
                slot_cols[:, scol:scol + klo[w]] = sc
                scol += klo[w]
            for w in grp:
                g, sl = pull_hi[(c, w)]
                n = khi[w] * P
                a = np.zeros(n, np.int64)
                a[:len(g)] = g
                hi_stream.append(a)
                m = np.arange(len(g))
                sc = np.full((P, khi[w]), -1.0, np.float32)
                sc[m % P, m // P] = sl
                slot_cols[:, scol:scol + khi[w]] = sc
                scol += khi[w]
        assert scol == npt
        idx_lo = _wrap_idx16(np.concatenate(lo_stream))
        idx_hi = _wrap_idx16(np.concatenate(hi_stream))

        push_stream = []
        pslot_cols = np.full((P, npp), -1.0, np.float32)
        scol = 0
        for i, (w, d) in enumerate(worder):
            k = c * len(worder) + i
            g = sloc_s[pstarts[k]:pends[k]]
            sl = slot_s[pstarts[k]:pends[k]]
            n = k2[i] * P
            a = np.zeros(n, np.int64)
            a[:len(g)] = g
            push_stream.append(a)
            m = np.arange(len(g))
            sc = np.full((P, k2[i]), -1.0, np.float32)
            sc[m % P, m // P] = sl
            pslot_cols[:, scol:scol + k2[i]] = sc
            scol += k2[i]
        assert scol == npp
        idx_push = _wrap_idx16(np.concatenate(push_stream))

        dinvd = np.zeros((P, NT), np.float32)
        lm = np.arange(NSH)
        dinvd[lm % P, lm // P] = dinv[c * NSH:(c + 1) * NSH]
        dinvbc = np.zeros(NSHP, np.float32)
        dinvbc[:NSH] = dinv[c * NSH:(c + 1) * NSH]
        dinvbc = np.broadcast_to(dinvbc.astype(BF16), (P, NSHP))

        cores.append({
            "idx_lo": idx_lo, "idx_hi": idx_hi, "idx_push": idx_push,
            "slot_pull": slot_cols, "slot_push": pslot_cols,
            "dinvd": dinvd, "dinvbc": np.ascontiguousarray(dinvbc),
        })

    layout = {
        "klo": klo, "khi": khi, "k2": k2,
        "pull_groups": pull_groups, "push_groups": push_groups,
        "worder": worder, "npt": npt, "npp": npp,
    }
    return {"x_pad": x_pad, "cores": cores, "layout": layout, "sig": sig}


# ---------------- meta/weights packing ----------------
# meta fp32 [128, npt + npp + NT + 2 + 256 + 16]
# wts bf16 [128, 256 W1 | 512 W2 | 32 W3 | 256 iota | 128 id | 6272 dinvbc]
def _pack_meta(core, W_np):
    npt = core["slot_pull"].shape[1]
    npp = core["slot_push"].shape[1]
    b1, b2, b3 = W_np["b1"], W_np["b2"], W_np["b3"]
    M = npt + npp + NT + 2 + 256 + 2 * NT * F_OUT_P
    meta = np.zeros((P, M), np.float32)
    o = 0
    meta[:, o:o + npt] = core["slot_pull"]; o += npt
    meta[:, o:o + npp] = core["slot_push"]; o += npp
    meta[:, o:o + NT] = core["dinvd"]; o += NT
    meta[:, o] = b2[np.arange(P)]
    meta[:, o + 1] = b2[128 + np.arange(P)]
    o += 2
    meta[:, o:o + 256] = b1[None, :]; o += 256
    # dinvd3: dinv[dst] repeated over the 16 output cols, per tile
    meta[:, o:o + NT * F_OUT_P] = np.repeat(
        core["dinvd"], F_OUT_P, axis=1).reshape(P, NT * F_OUT_P)
    o += NT * F_OUT_P
    b3r = np.zeros((P, NT * F_OUT_P), np.float32)
    b3r.reshape(P, NT, F_OUT_P)[:, :, 0:F_OUT] = b3[None, None, :]
    meta[:, o:o + NT * F_OUT_P] = b3r
    return meta


def _pack_wts(core, W_np):
    W1, W2, W3 = W_np["W1"], W_np["W2"], W_np["W3"]
    wts = np.zeros((P, 256 + 512 + 32 + 256 + 128 + NSHP), dtype=BF16)
    wts[:, 0:256] = W1.astype(BF16)
    wts[:, 256:512] = W2[0:128].astype(BF16)
    wts[:, 512:768] = W2[128:256].astype(BF16)
    wts[:, 768:784] = W3[0:128, :].astype(BF16)
    wts[:, 784:800] = W3[128:256, :].astype(BF16)
    wts[:, 800:1056] = np.arange(256, dtype=np.float32).astype(BF16)[None, :]
    wts[:, 1056:1184] = np.eye(128, dtype=np.float32).astype(BF16)
    wts[:, 1184:1184 + NSHP] = core["dinvbc"]
    return wts


def _build_program(layout):
    import concourse.bass as bass
    from concourse import bacc
    import concourse.mybir as mybir
    from concourse.tile import TileContext

    dt = mybir.dt
    Alu = mybir.AluOpType
    Act = mybir.ActivationFunctionType

    klo, khi, k2 = layout["klo"], layout["khi"], layout["k2"]
    pull_groups, push_groups = layout["pull_groups"], layout["push_groups"]
    worder = layout["worder"]
    npt, npp = layout["npt"], layout["npp"]
    M = npt + npp + NT + 2 + 256 + 2 * NT * F_OUT_P
    WCOLS = 256 + 512 + 32 + 256 + 128 + NSHP

    nc = bacc.Bacc(num_devices=NCORES)
    x_pad = nc.dram_tensor("x_pad", [NROWS, F_IN], dt.bfloat16, kind="ExternalInput")
    idx_lo_d = nc.dram_tensor("idx_lo", [P, int(klo.sum()) * 8], dt.int16, kind="ExternalInput")
    idx_hi_d = nc.dram_tensor("idx_hi", [P, int(khi.sum()) * 8], dt.int16, kind="ExternalInput")
    idx_push_d = nc.dram_tensor("idx_push", [P, int(k2.sum()) * 8], dt.int16, kind="ExternalInput")
    meta_d = nc.dram_tensor("meta", [P, M], dt.float32, kind="ExternalInput")
    wts_d = nc.dram_tensor("wts", [P, WCOLS], dt.bfloat16, kind="ExternalInput")
    out_d = nc.dram_tensor("out", [NSHP, F_OUT_P], dt.float32, kind="ExternalOutput")

    with TileContext(nc) as tc:
        with tc.tile_pool(name="const", bufs=1) as cpool, \
             tc.tile_pool(name="msgs", bufs=2) as mpool, \
             tc.tile_pool(name="work", bufs=3) as wpool, \
             tc.tile_pool(name="spool", bufs=1) as spool, \
             tc.tile_pool(name="persist", bufs=1) as ppool, \
             tc.tile_pool(name="ps", bufs=2, space="PSUM") as pspool, \
             tc.tile_pool(name="dram", bufs=1, space="DRAM") as dpool:

            idxlo_sb = cpool.tile([P, int(klo.sum()) * 8], dt.int16)
            nc.sync.dma_start(out=idxlo_sb[:], in_=idx_lo_d[:])
            idxhi_sb = cpool.tile([P, int(khi.sum()) * 8], dt.int16)
            nc.sync.dma_start(out=idxhi_sb[:], in_=idx_hi_d[:])
            idxp_sb = cpool.tile([P, int(k2.sum()) * 8], dt.int16)
            nc.sync.dma_start(out=idxp_sb[:], in_=idx_push_d[:])
            meta_sb = cpool.tile([P, M], dt.float32)
            nc.sync.dma_start(out=meta_sb[:], in_=meta_d[:])
            wts_sb = cpool.tile([P, WCOLS], dt.bfloat16)
            nc.sync.dma_start(out=wts_sb[:], in_=wts_d[:])

            slot_pull = meta_sb[:, 0:npt]
            slot_push = meta_sb[:, npt:npt + npp]
            dinvd = meta_sb[:, npt + npp:npt + npp + NT]
            b2c = meta_sb[:, npt + npp + NT:npt + npp + NT + 2]
            b1bc = meta_sb[:, npt + npp + NT + 2:npt + npp + NT + 2 + 256]
            _o3 = npt + npp + NT + 2 + 256
            dinvd3 = meta_sb[:, _o3:_o3 + NT * F_OUT_P]
            b3rep = meta_sb[:, _o3 + NT * F_OUT_P:M]
            w1_sb = wts_sb[:, 0:256]
            w2_sb = [wts_sb[:, 256:512], wts_sb[:, 512:768]]
            w3_sb = [wts_sb[:, 768:784], wts_sb[:, 784:800]]
            iota = wts_sb[:, 800:1056]
            ident = wts_sb[:, 1056:1184]
            dinvbc = wts_sb[:, 1184:1184 + NSHP]

            g1_d = dpool.tile([NSHP, F_HID], dt.bfloat16)
            y_d = dpool.tile([NSHP, 128], dt.bfloat16)
            pA = dpool.tile([NCORES * 256, ACOLS], dt.bfloat16)
            pB = dpool.tile([NCORES * 256, BCOLS], dt.bfloat16)
            aggA = dpool.tile([256, ACOLS], dt.bfloat16)
            aggB = dpool.tile([256, BCOLS], dt.bfloat16)
            p3_d = dpool.tile([NROWS, F_OUT_P], dt.float32)
            agg3_d = dpool.tile([NSHP, F_OUT_P], dt.float32)

            g1T_sb = ppool.tile([P, 2, NSHP], dt.bfloat16)
            ysb = ppool.tile([P, NT, F_OUT_P], dt.bfloat16)

            # ================= L1 PULL =================
            lo_off = 0
            hi_off = 0
            scol = 0
            for grp in pull_groups:
                nlo_g = int(sum(klo[w] for w in grp))
                nhi_g = int(sum(khi[w] for w in grp))
                ntg = nlo_g + nhi_g
                msl = mpool.tile([P, ntg, F_IN], dt.bfloat16, tag="msl",
                                 bufs=2, name=f"msl1_{grp[0]}")
                for off in range(0, nlo_g, 8):
                    ct = min(8, nlo_g - off)
                    nc.gpsimd.dma_gather(
                        out_ap=msl[:, off:off + ct, :], in_ap=x_pad[0:LO_LIM, :],
                        idxs_ap=idxlo_sb[:, (lo_off + off) * 8:
                                         (lo_off + off + ct) * 8],
                        num_idxs=ct * P, num_idxs_reg=ct * P,
                        elem_size=F_IN)
                for off in range(0, nhi_g, 8):
                    ct = min(8, nhi_g - off)
                    nc.gpsimd.dma_gather(
                        out_ap=msl[:, nlo_g + off:nlo_g + off + ct, :],
                        in_ap=x_pad[HI_OFF:HI_OFF + LO_LIM, :],
                        idxs_ap=idxhi_sb[:, (hi_off + off) * 8:
                                         (hi_off + off + ct) * 8],
                        num_idxs=ct * P, num_idxs_reg=ct * P,
                        elem_size=F_IN)
                g1stage = wpool.tile([P, 4, 256], dt.bfloat16, tag="g1stage",
                                     bufs=2, name=f"g1stage_{grp[0]}")
                # per-window buffer positions
                lo_rel = 0
                hi_rel = nlo_g
                sc_lo = scol
                sc_hi = scol + nlo_g
                for w in grp:
                    ww = _win_width(w)
                    agg = pspool.tile([P, ww], dt.float32, space="PSUM",
                                      tag="psA", bufs=3, name=f"agg1_{w}")
                    tiles = [(lo_rel + j, sc_lo + j) for j in range(int(klo[w]))] + \
                            [(hi_rel + j, sc_hi + j) for j in range(int(khi[w]))]
                    for jj, (bufpos, sc) in enumerate(tiles):
                        s_t = spool.tile([P, 256], dt.bfloat16, tag="s",
                                         bufs=16, name=f"s1_{w}_{jj}")
                        nc.vector.tensor_scalar(
                            out=s_t[:, 0:ww], in0=iota[:, 0:ww],
                            scalar1=slot_pull[:, sc:sc + 1], scalar2=None,
                            op0=Alu.is_equal)
                        nc.tensor.matmul(
                            agg[:], lhsT=msl[:, bufpos, :], rhs=s_t[:, 0:ww],
                            start=(jj == 0), stop=(jj == len(tiles) - 1))
                    lo_rel += int(klo[w]); sc_lo += int(klo[w])
                    hi_rel += int(khi[w]); sc_hi += int(khi[w])
                    aggsb = wpool.tile([P, ww], dt.bfloat16, tag="aggsb",
                                       bufs=3, name=f"aggsb_{w}")
                    nc.scalar.copy(out=aggsb[:], in_=agg[:])
                    nst = ww // P
                    for st in range(nst):
                        t_idx = w * 2 + st
                        zps = pspool.tile([P, 256], dt.float32, space="PSUM",
                                          tag="psB", bufs=3, name=f"z1_{t_idx}")
                        nc.tensor.matmul(zps[:], lhsT=aggsb[:, st * P:(st + 1) * P],
                                         rhs=w1_sb, start=True, stop=True)
                        t1 = wpool.tile([P, 256], dt.float32, tag="ep1",
                                        bufs=3, name=f"t1_{t_idx}")
                        nc.vector.tensor_scalar(
                            out=t1[:], in0=zps[:],
                            scalar1=dinvd[:, t_idx:t_idx + 1], scalar2=None,
                            op0=Alu.mult)
                        t2 = wpool.tile([P, 256], dt.float32, tag="ep2",
                                        bufs=3, name=f"t2_{t_idx}")
                        nc.vector.tensor_tensor(out=t2[:], in0=t1[:], in1=b1bc,
                                                op=Alu.add)
                        g1t = g1stage[:, t_idx - grp[0] * 2, :]
                        nc.scalar.activation(out=g1t, in_=t2[:], func=Act.Relu,
                                             scale=dinvd[:, t_idx:t_idx + 1])
                        for fc in range(2):
                            tp = pspool.tile([P, P], dt.bfloat16, space="PSUM",
                                             tag="psC", bufs=2,
                                             name=f"tp_{t_idx}_{fc}")
                            nc.tensor.transpose(tp[:], g1t[:, fc * P:(fc + 1) * P],
                                                ident)
                            nc.scalar.copy(
                                out=g1T_sb[:, fc, t_idx * P:(t_idx + 1) * P],
                                in_=tp[:])
                t0 = grp[0] * 2
                nst_g = sum(_win_width(w) // P for w in grp)
                nc.sync.dma_start(
                    out=g1_d[t0 * P:(t0 + nst_g) * P, :].rearrange(
                        "(t p) f -> p t f", p=P),
                    in_=g1stage[:, 0:nst_g, :])
                scol += ntg
                lo_off += nlo_g
                hi_off += nhi_g

            # ================= L2 PUSH =================
            k2pre = [0]
            for i in range(len(worder)):
                k2pre.append(k2pre[-1] + int(k2[i]))

            def push_stage(stage, gsrc, felem, w_lo, w_hi, rs_hook=None):
                """stage 2|3 over w blocks [w_lo, w_hi); one partial DMA per w."""
                for w in range(w_lo, w_hi):
                    ww = _win_width(w)
                    if stage == 2:
                        pstg = wpool.tile([P, 16, WW], dt.bfloat16, tag="p2st",
                                          bufs=2, name=f"p2st_{w}")
                    else:
                        pstg = wpool.tile([P, NCORES, 2, F_OUT_P], dt.float32,
                                          bufs=2, tag="p3st", name=f"p3st_{w}")
                    grp = push_groups[w]
                    p_off = k2pre[grp[0]]
                    ntg = k2pre[grp[-1] + 1] - p_off
                    msl = mpool.tile([P, ntg, felem], dt.bfloat16, tag="msl",
                                     bufs=2, name=f"msl{stage}_{grp[0]}")
                    for off in range(0, ntg, 8):
                        ct = min(8, ntg - off)
                        nc.gpsimd.dma_gather(
                            out_ap=msl[:, off:off + ct, :], in_ap=gsrc,
                            idxs_ap=idxp_sb[:, (p_off + off) * 8:
                                            (p_off + off + ct) * 8],
                            num_idxs=ct * P, num_idxs_reg=ct * P,
                            elem_size=felem)
                    if rs_hook is not None and w == rs_hook[0]:
                        rs_hook[1]()
                    rel = 0
                    for i in grp:
                        _, dcore = worder[i]
                        sc = k2pre[i]
                        ktiles = int(k2[i])
                        if stage == 2:
                            pps = [pspool.tile([P, ww], dt.float32, space="PSUM",
                                               tag=f"ps{chr(65+fc)}", bufs=3,
                                               name=f"p2_{i}_{fc}")
                                   for fc in range(2)]
                        else:
                            nh = ww // P
                            pps = [pspool.tile([P, F_OUT_P], dt.float32,
                                               space="PSUM",
                                               tag=f"ps{chr(65+h)}", bufs=3,
                                               name=f"p3_{i}_{h}")
                                   for h in range(nh)]
                        for jj in range(ktiles):
                            s_t = spool.tile([P, 256], dt.bfloat16, tag="s",
                                             bufs=16, name=f"s{stage}_{i}_{jj}")
                            nc.vector.tensor_scalar(
                                out=s_t[:, 0:ww], in0=iota[:, 0:ww],
                                scalar1=slot_push[:, sc + jj:sc + jj + 1],
                                scalar2=None, op0=Alu.is_equal)
                            first, last = (jj == 0), (jj == ktiles - 1)
                            if stage == 2:
                                for fc in range(2):
                                    nc.tensor.matmul(
                                        pps[fc][:],
                                        lhsT=msl[:, rel + jj,
                                                 fc * P:(fc + 1) * P],
                                        rhs=s_t[:, 0:ww],
                                        start=first, stop=last)
                            else:
                                for h in range(len(pps)):
                                    nc.tensor.matmul(
                                        pps[h][:],
                                        lhsT=s_t[:, h * P:(h + 1) * P],
                                        rhs=msl[:, rel + jj, 0:F_OUT_P],
                                        start=first, stop=last)
                        if stage == 2:
                            for fc in range(2):
                                nc.scalar.copy(
                                    out=pstg[:, dcore * 2 + fc, 0:ww],
                                    in_=pps[fc][:])
                        else:
                            for h in range(len(pps)):
                                nc.scalar.copy(out=pstg[:, dcore, h, :],
                                               in_=pps[h][:])
                        rel += ktiles
                    if stage == 2:
                        if w < ASPLIT:
                            tgt, col0 = pA, w * WW
                        else:
                            tgt, col0 = pB, (w - ASPLIT) * WW
                        nc.sync.dma_start(
                            out=tgt[:, col0:col0 + ww].rearrange(
                                "(a p) f -> p a f", p=P),
                            in_=pstg[:, :, 0:ww])
                    else:
                        nh = ww // P
                        for h in range(nh):
                            nc.sync.dma_start(
                                out=p3_d[:].rearrange(
                                    "(d r p) f -> p d r f", d=NCORES, p=P)[
                                    :, :, 2 * w + h, :],
                                in_=pstg[:, :, h, :])

            def _rs_a():
                nc.gpsimd.collective_compute(
                    "ReduceScatter", mybir.AluOpType.add,
                    replica_groups=[list(range(NCORES))],
                    ins=[pA[:].opt()], outs=[aggA[:].opt()])

            push_stage(2, g1_d[:], F_HID, 0, ASPLIT)
            push_stage(2, g1_d[:], F_HID, ASPLIT, NW, rs_hook=(18, _rs_a))

            # ================= z2 / y =================
            chunks = [(i * 512, 512) for i in range(12)] + [(6144, P)]

            def z2_chunk(c0, cw):
                a2 = wpool.tile([P, 2, 512], dt.bfloat16, tag="a2", bufs=2,
                                name=f"a2_{c0}")
                if c0 < ACOLS:
                    src = aggA[:, c0:c0 + cw]
                else:
                    src = aggB[:, c0 - ACOLS:c0 - ACOLS + cw]
                nc.sync.dma_start(
                    out=a2[:, :, 0:cw],
                    in_=src.rearrange("(a p) d -> p a d", p=P))
                g2 = wpool.tile([P, 2, 512], dt.bfloat16, tag="g2", bufs=2,
                                name=f"g2_{c0}")
                for fo in range(2):
                    zps = pspool.tile([P, 512], dt.float32, space="PSUM",
                                      tag="psA", bufs=3, name=f"z2_{c0}_{fo}")
                    for fi in range(2):
                        nc.tensor.matmul(
                            zps[:, 0:cw], lhsT=w2_sb[fi][:, fo * P:(fo + 1) * P],
                            rhs=a2[:, fi, 0:cw], start=(fi == 0), stop=False)
                    for fi in range(2):
                        nc.tensor.matmul(
                            zps[:, 0:cw], lhsT=w2_sb[fi][:, fo * P:(fo + 1) * P],
                            rhs=g1T_sb[:, fi, c0:c0 + cw],
                            start=False, stop=(fi == 1))
                    t1 = wpool.tile([P, 512], dt.float32, tag="zt1", bufs=2,
                                    name=f"zt1_{c0}_{fo}")
                    nc.vector.tensor_tensor(out=t1[:, 0:cw], in0=zps[:, 0:cw],
                                            in1=dinvbc[:, c0:c0 + cw],
                                            op=Alu.mult)
                    h2 = wpool.tile([P, 512], dt.float32, tag="zh", bufs=2,
                                    name=f"zh_{c0}_{fo}")
                    nc.vector.tensor_scalar(
                        out=h2[:, 0:cw], in0=t1[:, 0:cw],
                        scalar1=b2c[:, fo:fo + 1], scalar2=0.0,
                        op0=Alu.add, op1=Alu.max)
                    nc.vector.tensor_tensor(out=g2[:, fo, 0:cw], in0=h2[:, 0:cw],
                                            in1=dinvbc[:, c0:c0 + cw],
                                            op=Alu.mult)
                for t_idx in range(c0 // P, (c0 + cw) // P):
                    off = t_idx * P - c0
                    yps = pspool.tile([P, F_OUT_P], dt.float32, space="PSUM",
                                      tag="psB", bufs=3, name=f"y_{t_idx}")
                    for fi in range(2):
                        nc.tensor.matmul(yps[:],
                                         lhsT=g2[:, fi, off:off + P],
                                         rhs=w3_sb[fi],
                                         start=(fi == 0), stop=(fi == 1))
                    nc.vector.tensor_copy(out=ysb[:, t_idx, :], in_=yps[:])

            for (c0, cw) in chunks[:8]:
                z2_chunk(c0, cw)
            nc.gpsimd.collective_compute(
                "ReduceScatter", mybir.AluOpType.add,
                replica_groups=[list(range(NCORES))],
                ins=[pB[:].opt()], outs=[aggB[:].opt()])
            for (c0, cw) in chunks[8:]:
                z2_chunk(c0, cw)
            nc.sync.dma_start(
                out=y_d[0:ACOLS, 0:F_OUT_P].rearrange("(t p) f -> p t f", p=P),
                in_=ysb[:, 0:ACOLS // P, :])
            nc.sync.dma_start(
                out=y_d[ACOLS:NSHP, 0:F_OUT_P].rearrange(
                    "(t p) f -> p t f", p=P),
                in_=ysb[:, ACOLS // P:NT, :])

            # ================= L3 PUSH =================
            push_stage(3, y_d[:], 128, 0, NW)

            nc.gpsimd.collective_compute(
                "ReduceScatter", mybir.AluOpType.add,
                replica_groups=[list(range(NCORES))],
                ins=[p3_d[:].opt()], outs=[agg3_d[:].opt()])

            # ================= final epilogue (batched) =================
            agg3sb = ppool.tile([P, NT, F_OUT_P], dt.float32)
            nc.sync.dma_start(
                out=agg3sb[:],
                in_=agg3_d[:].rearrange("(t p) f -> p t f", p=P))
            outsb = ppool.tile([P, NT, F_OUT_P], dt.float32)
            a1 = wpool.tile([P, NT, F_OUT_P], dt.float32, tag="ftmp", bufs=2,
                            name="fa1")
            nc.vector.tensor_tensor(out=a1[:], in0=agg3sb[:], in1=ysb[:],
                                    op=Alu.add)
            t2b = wpool.tile([P, NT, F_OUT_P], dt.float32, tag="ftmp", bufs=2,
                             name="ft2b")
            nc.vector.tensor_tensor(
                out=t2b[:], in0=a1[:],
                in1=dinvd3.rearrange("p (t f) -> p t f", f=F_OUT_P),
                op=Alu.mult)
            t3b = ppool.tile([P, NT, F_OUT_P], dt.float32)
            nc.vector.tensor_tensor(
                out=t3b[:], in0=t2b[:],
                in1=b3rep.rearrange("p (t f) -> p t f", f=F_OUT_P),
                op=Alu.add)
            ex_all = ppool.tile([P, NT, F_OUT], dt.float32)
            nc.scalar.activation(out=ex_all[:], in_=t3b[:, :, 0:F_OUT],
                                 func=Act.Exp)
            sm_all = ppool.tile([P, NT], dt.float32)
            for t_idx in range(NT):
                nc.vector.tensor_reduce(
                    out=sm_all[:, t_idx:t_idx + 1], in_=ex_all[:, t_idx, :],
                    axis=mybir.AxisListType.X, op=Alu.add)
            ls_all = ppool.tile([P, NT], dt.float32)
            nc.scalar.activation(out=ls_all[:], in_=sm_all[:], func=Act.Ln)
            for t_idx in range(NT):
                nc.vector.tensor_scalar(
                    out=outsb[:, t_idx, 0:F_OUT], in0=t3b[:, t_idx, 0:F_OUT],
                    scalar1=ls_all[:, t_idx:t_idx + 1], scalar2=None,
                    op0=Alu.subtract)
            nc.sync.dma_start(
                out=out_d[:].rearrange("(t p) f -> p t f", p=P),
                in_=outsb[:])

    nc.finalize()
    return nc


_CACHE = {}


def kernel(x, edge_index, W1, b1, W2, b2, W3, b3):
    from concourse.bass_utils import run_bass_kernel_spmd

    prep = _preprocess(x, edge_index)
    layout = prep["layout"]

    if prep["sig"] not in _CACHE:
        _CACHE[prep["sig"]] = _build_program(layout)
    nc = _CACHE[prep["sig"]]

    W_np = {
        "W1": np.asarray(W1, np.float32), "b1": np.asarray(b1, np.float32),
        "W2": np.asarray(W2, np.float32), "b2": np.asarray(b2, np.float32),
        "W3": np.zeros((256, F_OUT_P), np.float32),
        "b3": np.asarray(b3, np.float32),
    }
    W_np["W3"][:, 0:F_OUT] = np.asarray(W3, np.float32)

    in_maps = []
    for c in range(NCORES):
        core = prep["cores"][c]
        in_maps.append({
            "x_pad": prep["x_pad"],
            "idx_lo": core["idx_lo"], "idx_hi": core["idx_hi"],
            "idx_push": core["idx_push"],
            "meta": _pack_meta(core, W_np),
            "wts": _pack_wts(core, W_np),
        })

    res = run_bass_kernel_spmd(nc, in_maps, core_ids=list(range(NCORES)))
    out = np.zeros((N, F_OUT), dtype=np.float32)
    for c in range(NCORES):
        out[c * NSH:(c + 1) * NSH] = res.results[c]["out"][:NSH, :F_OUT]
    return out
